# revision 21
# baseline (speedup 1.0000x reference)
"""GroupedQueryAttention on 8 Trainium2 NeuronCores via a Bass/Tile kernel.

Sharding (SPMD, one program on all 8 cores):
  core = (b, c) with b = core//4 (batch), c = core%4.
  Each core owns query rows x[b, c::4] (512 rows).  The stride-4 row
  interleave makes the causal-attention work identical on every core, so a
  single SPMD instruction stream load-balances perfectly.
  K/V are computed for each core's own rows and AllGathered across the
  4 cores of the batch (replica groups [[0..3],[4..7]]); gathered key order
  is a known permutation of 0..2047 handled by small DRAM mask tiles.

Layouts: projections are computed transposed (q^T/k^T = [dims, tokens]) so
attention probabilities come out of softmax already transposed for the PV
matmul; softmax denominators and RMSNorm sums (partition-axis reductions)
are done with ones-vector matmuls on the TensorEngine.  RoPE uses the
rotate-half form: the even/odd -> [evens; odds] permutation is folded into
Wq/Wk columns on the host; cos/sin (pre-scaled by the RMSNorm reciprocal on
device) are host-provided tables.

Falls back to a jax.jit implementation, then pure numpy, if the Bass path
is unavailable.
"""

import math
import os
import sys

import numpy as np

for _p in ("/opt/trn_rl_repo", "/root/.axon_site/_ro/trn_rl_repo"):
    if os.path.isdir(_p) and _p not in sys.path:
        sys.path.insert(0, _p)

D_MODEL = 2048
HQ = 16
HKV = 4
HEAD_DIM = 128
GROUP = 4
B, S = 2, 2048
RMS_EPS = 1.1920929e-07
ROPE_THETA = 10000.0
N_CORES = 8
P = 128
R = 512            # query rows per core
NDT = D_MODEL // P # 16 contraction tiles
NQC = 16           # q col tiles (== heads)
NKC = 4            # k col tiles (== kv heads)
NQT = R // P       # 4 q row tiles per core
SCALE = 1.0 / math.sqrt(HEAD_DIM)

_BF16 = None  # ml_dtypes.bfloat16, set on first use


# ---------------------------------------------------------------------------
# Bass kernel builder
# ---------------------------------------------------------------------------

def _build_nc():
    import concourse.bacc as bacc
    import concourse.bass as bass
    import concourse.tile as tile
    from concourse import mybir
    from concourse.alu_op_type import AluOpType as Alu

    f32 = mybir.dt.float32
    bf16 = mybir.dt.bfloat16
    Exp = mybir.ActivationFunctionType.Exp
    Sqrt = mybir.ActivationFunctionType.Sqrt
    Square = mybir.ActivationFunctionType.Square

    nc = bacc.Bacc("TRN2", debug=False, enable_asserts=False,
                   target_bir_lowering=False, num_devices=N_CORES)

    # ---- I/O ----
    xT = nc.dram_tensor("xT", [D_MODEL, R], bf16, kind="ExternalInput")
    wq = nc.dram_tensor("wq", [NQC, P, NDT, P], bf16, kind="ExternalInput")
    wk = nc.dram_tensor("wk", [NKC, P, NDT, P], bf16, kind="ExternalInput")
    wv = nc.dram_tensor("wv", [D_MODEL, 512], bf16, kind="ExternalInput")
    wo = nc.dram_tensor("wo", [D_MODEL, D_MODEL], bf16, kind="ExternalInput")
    bq = nc.dram_tensor("bq", [NQC, P], f32, kind="ExternalInput")
    qn = nc.dram_tensor("qn", [NQC, P], f32, kind="ExternalInput")
    bk = nc.dram_tensor("bk", [NKC, P], f32, kind="ExternalInput")
    kn = nc.dram_tensor("kn", [NKC, P], f32, kind="ExternalInput")
    bvr = nc.dram_tensor("bvr", [P, 512], f32, kind="ExternalInput")
    bor = nc.dram_tensor("bor", [P, D_MODEL], f32, kind="ExternalInput")
    gate = nc.dram_tensor("gate", [1, HQ], f32, kind="ExternalInput")
    cs = nc.dram_tensor("cs", [2, 64, R], f32, kind="ExternalInput")
    msk = nc.dram_tensor("msk", [P, 4, P], bf16, kind="ExternalInput")
    out = nc.dram_tensor("out", [R, D_MODEL], f32, kind="ExternalOutput")

    cc_in = nc.dram_tensor("cc_in", [1024, 512], bf16, kind="Internal")
    cc_out = nc.dram_tensor("cc_out", [4, 1024, 512], bf16, kind="Internal")

    from contextlib import ExitStack

    with tile.TileContext(nc) as tc, ExitStack() as stk:
        res = stk.enter_context(tc.tile_pool(name="res", bufs=1))
        stream = stk.enter_context(tc.tile_pool(name="stream", bufs=2))
        tmp = stk.enter_context(tc.tile_pool(name="tmp", bufs=4))
        small = stk.enter_context(tc.tile_pool(name="small", bufs=2))
        cs_pool = stk.enter_context(tc.tile_pool(name="csp", bufs=1))
        early_stk = ExitStack()
        early = early_stk.enter_context(tc.tile_pool(name="early", bufs=1))

        if True:
            # ---- resident loads ----
            xT_sb = early.tile([P, NDT, R], bf16)
            nc.sync.dma_start(
                out=xT_sb, in_=xT.rearrange("(dt p) r -> p dt r", p=P))
            wv_sb = early.tile([P, NDT, 512], bf16)
            nc.sync.dma_start(
                out=wv_sb, in_=wv.rearrange("(dt p) v -> p dt v", p=P))
            bq_sb = res.tile([P, NQC], f32)
            nc.sync.dma_start(out=bq_sb, in_=bq.rearrange("a p -> p a"))
            qn_sb = res.tile([P, NQC], f32)
            nc.sync.dma_start(out=qn_sb, in_=qn.rearrange("a p -> p a"))
            bk_sb = res.tile([P, NKC], f32)
            nc.sync.dma_start(out=bk_sb, in_=bk.rearrange("a p -> p a"))
            kn_sb = res.tile([P, NKC], f32)
            nc.sync.dma_start(out=kn_sb, in_=kn.rearrange("a p -> p a"))
            bvr_sb = res.tile([P, 512], f32)
            nc.sync.dma_start(out=bvr_sb, in_=bvr[:, :])
            bor_sb = res.tile([P, 4, 512], f32)
            nc.sync.dma_start(
                out=bor_sb, in_=bor.rearrange("p (o c) -> p o c", c=512))
            gate_sb = res.tile([1, HQ], f32)
            nc.sync.dma_start(out=gate_sb, in_=gate[:, :])
            cs_sb = res.tile([64, 2, R], f32)
            nc.sync.dma_start(out=cs_sb, in_=cs.rearrange("s p t -> p s t"))
            msk_sb = res.tile([P, 4, P], bf16)
            nc.sync.dma_start(out=msk_sb, in_=msk[:, :, :])

            ones_sb = res.tile([P, 1], bf16)
            nc.vector.memset(ones_sb, 1.0)
            onesrow_sb = res.tile([1, P], f32)
            nc.vector.memset(onesrow_sb, 1.0)
            eps_sb = res.tile([1, 1], f32)
            nc.vector.memset(eps_sb, RMS_EPS)

            qT = res.tile([P, NQC, R], bf16)
            kTl = res.tile([P, NKC, R], bf16)
            vl = res.tile([P, 4, R], bf16)
            kT_f = res.tile([P, HKV, 4, 512], bf16)   # [d, h4, r, i]
            v_f = res.tile([P, 4, 4, 512], bf16)      # [keys, r, w, vcol]
            attnT = res.tile([P, HQ, R], bf16)

            ps_stack = ExitStack()
            ps_proj = ps_stack.enter_context(
                tc.tile_pool(name="ps_proj", bufs=3, space="PSUM"))
            ps_small = ps_stack.enter_context(
                tc.tile_pool(name="ps_small", bufs=2, space="PSUM"))

            def finish_rms(sq_ps, inv_n):
                """[1,R] f32 = 1/sqrt(mean(col sums of squares)+eps)."""
                tmp_sb = small.tile([1, R], f32, tag="rms_tmp", bufs=1)
                nc.scalar.activation(tmp_sb, sq_ps, Sqrt,
                                     bias=eps_sb[:, 0:1], scale=inv_n)
                dst_sb = small.tile([1, R], f32, tag="rkq", bufs=1)
                nc.vector.reciprocal(dst_sb, tmp_sb)
                return dst_sb

            def build_cs(r_sb):
                """c1 = [cos*r; sin*r], c2 = [-sin*r; cos*r]."""
                rb_ps = ps_proj.tile([P, R], f32, tag="proj")
                nc.tensor.matmul(rb_ps, onesrow_sb, r_sb,
                                 skip_group_check=True)
                rb_sb = tmp.tile([P, R], f32, tag="rb", bufs=2)
                nc.scalar.copy(rb_sb, rb_ps)
                c1 = cs_pool.tile([P, R], f32, tag="cs1", name="cs1")
                c2 = cs_pool.tile([P, R], f32, tag="cs2", name="cs2")
                nc.vector.tensor_tensor(c1[0:64, :], cs_sb[:, 0, :],
                                        rb_sb[0:64, :], Alu.mult)
                nc.vector.tensor_tensor(c1[64:128, :], cs_sb[:, 1, :],
                                        rb_sb[64:128, :], Alu.mult)
                nc.vector.tensor_scalar(c2[0:64, :], c1[64:128, :],
                                        -1.0, None, Alu.mult)
                nc.vector.tensor_copy(c2[64:128, :], c1[0:64, :])
                return c1, c2

            def rope_unit(dst, c1, c2):
                """In-place rotate-half RoPE on dst [128, R] bf16."""
                a = tmp.tile([P, R], bf16, tag="ropeA", bufs=2)
                b2 = tmp.tile([P, R], bf16, tag="ropeB", bufs=2)
                p1 = tmp.tile([P, R], bf16, tag="ropeP", bufs=2)
                q1 = tmp.tile([P, R], bf16, tag="ropeQ", bufs=2)
                nc.gpsimd.tensor_copy(a[0:64, :], dst[0:64, :])
                nc.gpsimd.tensor_copy(a[64:128, :], dst[0:64, :])
                nc.scalar.copy(b2[0:64, :], dst[64:128, :])
                nc.scalar.copy(b2[64:128, :], dst[64:128, :])
                nc.vector.tensor_tensor(p1, a, c1, Alu.mult)
                nc.vector.tensor_tensor(q1, b2, c2, Alu.mult)
                nc.vector.tensor_tensor(dst, p1, q1, Alu.add)

            # ---------------- K projection ----------------
            ksq_ps = ps_small.tile([1, R], f32, name="ksq_ps")
            for kc in range(NKC):
                wk_t = stream.tile([P, NDT, P], bf16, tag="w", name="wk_t")
                nc.sync.dma_start(out=wk_t, in_=wk[kc])
                ps = ps_proj.tile([P, R], f32, tag="proj")
                for dt in range(NDT):
                    nc.tensor.matmul(ps, wk_t[:, dt, :], xT_sb[:, dt, :],
                                     start=(dt == 0), stop=(dt == NDT - 1))
                zsq = tmp.tile([P, R], bf16, tag="zsq")
                nc.scalar.activation(zsq, ps, Square,
                                     bias=bk_sb[:, kc:kc + 1], scale=1.0)
                nc.tensor.matmul(ksq_ps, ones_sb, zsq,
                                 start=(kc == 0), stop=(kc == NKC - 1),
                                 skip_group_check=True)
                nc.vector.tensor_scalar(kTl[:, kc, :], ps,
                                        bk_sb[:, kc:kc + 1],
                                        kn_sb[:, kc:kc + 1],
                                        Alu.add, Alu.mult)
            rk_sb = finish_rms(ksq_ps, 1.0 / 512)
            cs1k, cs2k = build_cs(rk_sb)
            for kc in range(NKC):
                rope_unit(kTl[:, kc, :], cs1k, cs2k)
            nc.sync.dma_start(
                out=cc_in[0:512, :].rearrange("(kc p) t -> p kc t", p=P),
                in_=kTl)

            # ---------------- V projection ----------------
            for it in range(4):
                ps = ps_proj.tile([P, 512], f32, tag="proj")
                for dt in range(NDT):
                    nc.tensor.matmul(ps, xT_sb[:, dt, it * P:(it + 1) * P],
                                     wv_sb[:, dt, :],
                                     start=(dt == 0), stop=(dt == NDT - 1))
                nc.vector.tensor_tensor(vl[:, it, :], ps, bvr_sb, Alu.add)
            nc.sync.dma_start(
                out=cc_in[512:1024, :].rearrange("(it p) t -> p it t", p=P),
                in_=vl)

            # ---------------- AllGather K/V ----------------
            nc.gpsimd.collective_compute(
                "AllGather", Alu.bypass,
                replica_groups=[[0, 1, 2, 3], [4, 5, 6, 7]],
                ins=[cc_in[:, :]], outs=[cc_out[:, :, :]])
            for r4 in range(4):
                nc.sync.dma_start(
                    out=kT_f[:, :, r4, :],
                    in_=cc_out[r4, 0:512, :].rearrange("(h p) t -> p h t", p=P))
                nc.sync.dma_start(
                    out=v_f[:, r4, :, :],
                    in_=cc_out[r4, 512:1024, :].rearrange("(w p) t -> p w t", p=P))

            # ---------------- Q projection ----------------
            qsq_ps = ps_small.tile([1, R], f32, name="qsq_ps")
            for qc in range(NQC):
                wq_t = stream.tile([P, NDT, P], bf16, tag="w", name="wq_t")
                nc.sync.dma_start(out=wq_t, in_=wq[qc])
                ps = ps_proj.tile([P, R], f32, tag="proj")
                for dt in range(NDT):
                    nc.tensor.matmul(ps, wq_t[:, dt, :], xT_sb[:, dt, :],
                                     start=(dt == 0), stop=(dt == NDT - 1))
                zsq = tmp.tile([P, R], bf16, tag="zsq")
                nc.scalar.activation(zsq, ps, Square,
                                     bias=bq_sb[:, qc:qc + 1], scale=1.0)
                nc.tensor.matmul(qsq_ps, ones_sb, zsq,
                                 start=(qc == 0), stop=(qc == NQC - 1),
                                 skip_group_check=True)
                nc.vector.tensor_scalar(qT[:, qc, :], ps,
                                        bq_sb[:, qc:qc + 1],
                                        qn_sb[:, qc:qc + 1],
                                        Alu.add, Alu.mult)
            rq_sb = finish_rms(qsq_ps, 1.0 / D_MODEL)
            cs1q, cs2q = build_cs(rq_sb)
            for qc in range(NQC):
                rope_unit(qT[:, qc, :], cs1q, cs2q)

            ps_stack.close()
            early_stk.close()
            wo_pool = stk.enter_context(tc.tile_pool(name="wop", bufs=3))
            probs_pool = stk.enter_context(tc.tile_pool(name="probs", bufs=4))
            grecip_pool = stk.enter_context(tc.tile_pool(name="grec", bufs=2))
            outsb_pool = stk.enter_context(tc.tile_pool(name="outsb", bufs=4))

            # ---------------- Attention ----------------
            with ExitStack() as att_stk:
                ps_sc = att_stk.enter_context(
                    tc.tile_pool(name="ps_sc", bufs=2, space="PSUM"))
                ps_at = att_stk.enter_context(
                    tc.tile_pool(name="ps_at", bufs=2, space="PSUM"))
                ps_sm = att_stk.enter_context(
                    tc.tile_pool(name="ps_sm", bufs=1, space="PSUM"))
                ps_rp = att_stk.enter_context(
                    tc.tile_pool(name="ps_rp", bufs=1, space="PSUM"))
                for h in range(HQ):
                    h4 = h // GROUP
                    at_ps = ps_at.tile([P, R], f32)
                    sm_ps = ps_sm.tile([1, R], f32)
                    for t in range(NQT):
                        off = P * t
                        for pair in range(2):
                            sc_ps = ps_sc.tile([P, 2, 512], f32)
                            for i2 in range(2):
                                r4 = 2 * pair + i2
                                nc.tensor.matmul(
                                    sc_ps[:, i2, off:512],
                                    kT_f[:, h4, r4, off:off + P],
                                    qT[:, h, off:512])
                            probs = probs_pool.tile([P, 2, 512], bf16)
                            nc.scalar.activation(
                                probs[:, :, off:512], sc_ps[:, :, off:512],
                                Exp, bias=0.0, scale=SCALE)
                            nc.vector.tensor_tensor(
                                probs[:, :, off:off + P],
                                probs[:, :, off:off + P],
                                msk_sb[:, 2 * pair:2 * pair + 2, :], Alu.mult)
                            for i2 in range(2):
                                r4 = 2 * pair + i2
                                first = (t == 0 and pair == 0 and i2 == 0)
                                last = (t == NQT - 1 and pair == 1 and i2 == 1)
                                nc.tensor.matmul(
                                    sm_ps[0:1, off:512], ones_sb,
                                    probs[:, i2, off:512],
                                    start=first, stop=last,
                                    skip_group_check=True)
                                nc.tensor.matmul(
                                    at_ps[:, off:512],
                                    v_f[:, r4, t, h4 * P:(h4 + 1) * P],
                                    probs[:, i2, off:512],
                                    start=first, stop=last,
                                    skip_group_check=True)
                    sums_sb = small.tile([1, R], f32, tag="sums")
                    nc.vector.tensor_copy(sums_sb, sm_ps)
                    recip_sb = small.tile([1, R], f32, tag="recip")
                    nc.vector.reciprocal(recip_sb, sums_sb)
                    nc.vector.tensor_scalar(recip_sb, recip_sb,
                                            gate_sb[0:1, h:h + 1], None,
                                            Alu.mult)
                    rp_ps = ps_rp.tile([P, R], f32)
                    nc.tensor.matmul(rp_ps, onesrow_sb, recip_sb,
                                     skip_group_check=True)
                    grecip = grecip_pool.tile([P, R], f32)
                    nc.scalar.copy(grecip, rp_ps)
                    nc.vector.tensor_tensor(attnT[:, h, :], at_ps, grecip,
                                            Alu.mult)

            # ---------------- Output projection ----------------
            with tc.tile_pool(name="ps_o", bufs=2, space="PSUM") as ps_o:
                for oc in range(4):
                    ps_list = [ps_o.tile([P, 512], f32, tag=f"o{rt}",
                                         name=f"ops{rt}")
                               for rt in range(NQT)]
                    for hd in range(HQ):
                        wo_t = wo_pool.tile([P, 512], bf16, tag="wo")
                        nc.sync.dma_start(
                            out=wo_t,
                            in_=wo[hd * P:(hd + 1) * P, oc * 512:(oc + 1) * 512])
                        for rt in range(NQT):
                            nc.tensor.matmul(
                                ps_list[rt], attnT[:, hd, rt * P:(rt + 1) * P],
                                wo_t, start=(hd == 0), stop=(hd == HQ - 1),
                                skip_group_check=True)
                    for rt in range(NQT):
                        o_sb = outsb_pool.tile([P, 512], f32)
                        nc.vector.tensor_tensor(o_sb, ps_list[rt],
                                                bor_sb[:, oc, :], Alu.add)
                        nc.sync.dma_start(
                            out=out[rt * P:(rt + 1) * P,
                                    oc * 512:(oc + 1) * 512],
                            in_=o_sb)

    nc.compile()
    return nc


_NC_CACHE = None


def _get_nc():
    global _NC_CACHE
    if _NC_CACHE is None:
        _NC_CACHE = _build_nc()
    return _NC_CACHE


# ---------------------------------------------------------------------------
# Host-side preparation
# ---------------------------------------------------------------------------

_PREP_CACHE = {}


def _perm():
    """Even/odd -> [evens; odds] permutation within each 128-dim head."""
    p = np.arange(HEAD_DIM).reshape(64, 2).T.reshape(-1)  # [0,2,..,126,1,3,..]
    return p


def _prep_static(Wq, bq, Wk, bk, Wv, bv, Wo, bo, qn_w, kn_w, gate_logits):
    """Weight-dependent, call-invariant prep (cached by array ids)."""
    global _BF16
    key = tuple(id(a) for a in (Wq, bq, Wk, bk, Wv, bv, Wo, bo, qn_w, kn_w,
                                gate_logits))
    hit = _PREP_CACHE.get(key)
    if hit is not None:
        return hit
    import ml_dtypes
    _BF16 = ml_dtypes.bfloat16

    pm = _perm()
    qperm = (np.arange(HQ)[:, None] * HEAD_DIM + pm[None, :]).reshape(-1)
    kperm = (np.arange(HKV)[:, None] * HEAD_DIM + pm[None, :]).reshape(-1)

    Wq_p = np.ascontiguousarray(Wq[:, qperm])
    Wk_p = np.ascontiguousarray(Wk[:, kperm])
    # [qc][p (d within tile)][dt][q (col within tile)]
    wq_t = np.ascontiguousarray(
        Wq_p.astype(_BF16).reshape(NDT, P, NQC, P).transpose(2, 1, 0, 3))
    wk_t = np.ascontiguousarray(
        Wk_p.astype(_BF16).reshape(NDT, P, NKC, P).transpose(2, 1, 0, 3))
    wv_b = np.ascontiguousarray(Wv.astype(_BF16))
    wo_b = np.ascontiguousarray(Wo.astype(_BF16))

    bq_p = np.ascontiguousarray(bq[qperm].astype(np.float32).reshape(NQC, P))
    qn_p = np.ascontiguousarray(qn_w[qperm].astype(np.float32).reshape(NQC, P))
    bk_p = np.ascontiguousarray(bk[kperm].astype(np.float32).reshape(NKC, P))
    kn_p = np.ascontiguousarray(kn_w[kperm].astype(np.float32).reshape(NKC, P))
    bvr = np.ascontiguousarray(
        np.broadcast_to(bv.astype(np.float32)[None, :], (P, 512)))
    bor = np.ascontiguousarray(
        np.broadcast_to(bo.astype(np.float32)[None, :], (P, D_MODEL)))
    gates = 1.0 / (1.0 + np.exp(-gate_logits.astype(np.float32)))
    gate_t = np.ascontiguousarray(gates.reshape(1, HQ).astype(np.float32))

    ent = dict(wq=wq_t, wk=wk_t, wv=wv_b, wo=wo_b, bq=bq_p, qn=qn_p,
               bk=bk_p, kn=kn_p, bvr=bvr, bor=bor, gate=gate_t,
               _refs=(Wq, bq, Wk, bk, Wv, bv, Wo, bo, qn_w, kn_w, gate_logits))
    _PREP_CACHE.clear()
    _PREP_CACHE[key] = ent
    return ent


def _cs_table(c, sp):
    """cos/sin for positions sp + 4*i + c, i in 0..R-1. [2, 64, R] f32."""
    inv = 1.0 / (ROPE_THETA ** (np.arange(64, dtype=np.float64) / 64.0))
    pos = sp + 4.0 * np.arange(R, dtype=np.float64) + c
    ang = pos[None, :] * inv[:, None]              # [64, R]
    return np.ascontiguousarray(
        np.stack([np.cos(ang), np.sin(ang)]).astype(np.float32))


def _mask_tiles(c):
    """msk[k, r, j] = 1 if key (4k+r) <= query (4j+c) else 0; bf16."""
    k = np.arange(P)[:, None, None]
    r = np.arange(4)[None, :, None]
    j = np.arange(P)[None, None, :]
    return np.ascontiguousarray(
        ((4 * k + r) <= (4 * j + c)).astype(_BF16))


def _is_tril(mask):
    m = np.asarray(mask)
    if m.shape != (S, S):
        return False
    idx = np.arange(S)
    # spot-check: full check is 4M elements, cheap enough once
    return bool(np.array_equal(m, idx[None, :] <= idx[:, None]))


_TRIL_CACHE = {}


# ---------------------------------------------------------------------------
# Fallback implementations (numpy / jax.jit)
# ---------------------------------------------------------------------------

def _np_rmsnorm(x, w):
    var = np.mean(np.square(x), axis=-1, keepdims=True)
    return x * (1.0 / np.sqrt(var + RMS_EPS)) * w


def _np_rope(x, positions):
    half = x.shape[-1] // 2
    inv_freq = 1.0 / (ROPE_THETA ** (np.arange(half, dtype=np.float32) / half))
    ang = positions.astype(np.float32)[:, None] * inv_freq[None, :]
    cos, sin = np.cos(ang), np.sin(ang)
    while cos.ndim < x.ndim:
        cos, sin = cos[None], sin[None]
    x1, x2 = x[..., 0::2], x[..., 1::2]
    outp = np.empty_like(x)
    outp[..., 0::2] = x1 * cos - x2 * sin
    outp[..., 1::2] = x1 * sin + x2 * cos
    return outp


def _np_reference(x, Wq, bq, Wk, bk, Wv, bv, Wo, bo, qn_w, kn_w,
                  gate_logits, mask, sp):
    outp = np.empty((B, S, D_MODEL), dtype=np.float32)
    positions = sp + np.arange(S)
    gates = 1.0 / (1.0 + np.exp(-gate_logits.astype(np.float32)))
    for b in range(B):
        q = _np_rmsnorm(x[b] @ Wq + bq, qn_w)
        k = _np_rmsnorm(x[b] @ Wk + bk, kn_w)
        v = x[b] @ Wv + bv
        q = q.reshape(S, HQ, HEAD_DIM).transpose(1, 0, 2)
        k = k.reshape(S, HKV, HEAD_DIM).transpose(1, 0, 2)
        v = v.reshape(S, HKV, HEAD_DIM).transpose(1, 0, 2)
        q = _np_rope(q, positions)
        k = _np_rope(k, positions)
        attn = np.empty((S, HQ, HEAD_DIM), dtype=np.float32)
        for h in range(HQ):
            g = h // GROUP
            s = (q[h] @ k[g].T) * SCALE
            s = np.where(mask, s, -np.inf).astype(np.float32)
            s -= s.max(axis=-1, keepdims=True)
            p = np.exp(s)
            p /= p.sum(axis=-1, keepdims=True)
            attn[:, h, :] = (p @ v[g]) * gates[h]
        outp[b] = attn.reshape(S, D_MODEL) @ Wo + bo
    return outp


# ---------------------------------------------------------------------------
# Entry point
# ---------------------------------------------------------------------------

LAST_EXEC_NS = None


def kernel(x, Wq, bq, Wk, bk, Wv, bv, Wo, bo, qn_w, kn_w,
           gate_logits, mask, start_pos, **_ignored):
    global LAST_EXEC_NS
    x = np.asarray(x, dtype=np.float32)
    Wq = np.asarray(Wq, dtype=np.float32); bq = np.asarray(bq, dtype=np.float32)
    Wk = np.asarray(Wk, dtype=np.float32); bk = np.asarray(bk, dtype=np.float32)
    Wv = np.asarray(Wv, dtype=np.float32); bv = np.asarray(bv, dtype=np.float32)
    Wo = np.asarray(Wo, dtype=np.float32); bo = np.asarray(bo, dtype=np.float32)
    qn_w = np.asarray(qn_w, dtype=np.float32)
    kn_w = np.asarray(kn_w, dtype=np.float32)
    gate_logits = np.asarray(gate_logits, dtype=np.float32)
    mask = np.asarray(mask)
    sp = int(np.asarray(start_pos))

    if not os.environ.get("GQA_NO_DEVICE"):
        mk = id(mask)
        if mk not in _TRIL_CACHE:
            _TRIL_CACHE.clear()
            _TRIL_CACHE[mk] = (_is_tril(mask), mask)
        if _TRIL_CACHE[mk][0] and x.shape == (B, S, D_MODEL):
            try:
                return _bass_path(x, Wq, bq, Wk, bk, Wv, bv, Wo, bo,
                                  qn_w, kn_w, gate_logits, sp)
            except Exception:
                import traceback
                traceback.print_exc()

    return _np_reference(x, Wq, bq, Wk, bk, Wv, bv, Wo, bo, qn_w, kn_w,
                         gate_logits, mask, sp)


def _bass_path(x, Wq, bq, Wk, bk, Wv, bv, Wo, bo, qn_w, kn_w,
               gate_logits, sp):
    global LAST_EXEC_NS
    from concourse.bass_utils import run_bass_kernel_spmd

    st = _prep_static(Wq, bq, Wk, bk, Wv, bv, Wo, bo, qn_w, kn_w, gate_logits)
    static = {k: v for k, v in st.items() if not k.startswith("_")}

    in_maps = []
    for core in range(N_CORES):
        b, c = divmod(core, 4)
        xT_own = np.ascontiguousarray(x[b, c::4, :].T.astype(_BF16))
        m = dict(static)
        m["xT"] = xT_own
        m["cs"] = _cs_table(c, sp)
        m["msk"] = _mask_tiles(c)
        in_maps.append(m)

    trace = bool(os.environ.get("GQA_TRACE"))
    try:
        res = run_bass_kernel_spmd(_get_nc(), in_maps,
                                   core_ids=list(range(N_CORES)), trace=trace)
    except Exception:
        if not trace:
            raise
        res = run_bass_kernel_spmd(_get_nc(), in_maps,
                                   core_ids=list(range(N_CORES)), trace=False)
    LAST_EXEC_NS = res.exec_time_ns

    outp = np.empty((B, S, D_MODEL), dtype=np.float32)
    for core in range(N_CORES):
        b, c = divmod(core, 4)
        outp[b, c::4, :] = res.results[core]["out"]
    return outp


# revision 65
# speedup vs baseline: 8920.7568x; 8920.7568x over previous
"""GroupedQueryAttention on 8 Trainium2 NeuronCores via a Bass/Tile kernel.

Sharding (SPMD, one program on all 8 cores):
  core = (b, c) with b = core//4 (batch), c = core%4.
  Each core owns query rows x[b, c::4] (512 rows).  The stride-4 row
  interleave makes the causal-attention work identical on every core, so a
  single SPMD instruction stream load-balances perfectly.
  K/V are computed for each core's own rows and AllGathered across the
  4 cores of the batch (replica groups [[0..3],[4..7]]); gathered key order
  is a known permutation of 0..2047 handled by small DRAM mask tiles.

Layouts: projections are computed transposed (q^T/k^T = [dims, tokens]) so
attention probabilities come out of softmax already transposed for the PV
matmul; softmax denominators and RMSNorm sums (partition-axis reductions)
are done with ones-vector matmuls on the TensorEngine.  RoPE uses the
rotate-half form: the even/odd -> [evens; odds] permutation is folded into
Wq/Wk columns on the host; cos/sin (pre-scaled by the RMSNorm reciprocal on
device) are host-provided tables.

Falls back to a jax.jit implementation, then pure numpy, if the Bass path
is unavailable.
"""

import math
import os
import sys

import numpy as np

for _p in ("/opt/trn_rl_repo", "/root/.axon_site/_ro/trn_rl_repo"):
    if os.path.isdir(_p) and _p not in sys.path:
        sys.path.insert(0, _p)

D_MODEL = 2048
HQ = 16
HKV = 4
HEAD_DIM = 128
GROUP = 4
B, S = 2, 2048
RMS_EPS = 1.1920929e-07
ROPE_THETA = 10000.0
N_CORES = 8
P = 128
R = 512            # query rows per core
NDT = D_MODEL // P # 16 contraction tiles
NQC = 16           # q col tiles (== heads)
NKC = 4            # k col tiles (== kv heads)
NQT = R // P       # 4 q row tiles per core
SCALE = 1.0 / math.sqrt(HEAD_DIM)

_BF16 = None  # ml_dtypes.bfloat16, set on first use


# ---------------------------------------------------------------------------
# Bass kernel builder
# ---------------------------------------------------------------------------

def _build_nc():
    import concourse.bacc as bacc
    import concourse.bass as bass
    import concourse.tile as tile
    from concourse import mybir
    from concourse.alu_op_type import AluOpType as Alu

    f32 = mybir.dt.float32
    bf16 = mybir.dt.bfloat16
    Exp = mybir.ActivationFunctionType.Exp
    Sqrt = mybir.ActivationFunctionType.Sqrt
    Square = mybir.ActivationFunctionType.Square

    nc = bacc.Bacc("TRN2", debug=False, enable_asserts=False,
                   target_bir_lowering=False, num_devices=N_CORES)

    # ---- I/O ----
    xT = nc.dram_tensor("xT", [D_MODEL, R], bf16, kind="ExternalInput")
    wq = nc.dram_tensor("wq", [NQC, P, NDT, P], bf16, kind="ExternalInput")
    wk = nc.dram_tensor("wk", [NKC, P, NDT, P], bf16, kind="ExternalInput")
    wv = nc.dram_tensor("wv", [D_MODEL, 512], bf16, kind="ExternalInput")
    wo = nc.dram_tensor("wo", [D_MODEL, D_MODEL], bf16, kind="ExternalInput")
    bq = nc.dram_tensor("bq", [NQC, P], f32, kind="ExternalInput")
    qn = nc.dram_tensor("qn", [NQC, P], f32, kind="ExternalInput")
    bk = nc.dram_tensor("bk", [NKC, P], f32, kind="ExternalInput")
    kn = nc.dram_tensor("kn", [NKC, P], f32, kind="ExternalInput")
    bvr = nc.dram_tensor("bvr", [P, 512], f32, kind="ExternalInput")
    bor = nc.dram_tensor("bor", [P, D_MODEL], f32, kind="ExternalInput")
    gate = nc.dram_tensor("gate", [1, HQ], f32, kind="ExternalInput")
    cs = nc.dram_tensor("cs", [2, 64, R], f32, kind="ExternalInput")
    msk = nc.dram_tensor("msk", [P, 4, P], bf16, kind="ExternalInput")
    out = nc.dram_tensor("out", [R, D_MODEL], f32, kind="ExternalOutput")

    cc_in = nc.dram_tensor("cc_in", [1024, 512], bf16, kind="Internal")
    cc_out = nc.dram_tensor("cc_out", [4, 1024, 512], bf16, kind="Internal")

    from contextlib import ExitStack

    with tile.TileContext(nc) as tc, ExitStack() as stk:
        res = stk.enter_context(tc.tile_pool(name="res", bufs=1))
        stream = stk.enter_context(tc.tile_pool(name="stream", bufs=2))
        tmp = stk.enter_context(tc.tile_pool(name="tmp", bufs=4))
        small = stk.enter_context(tc.tile_pool(name="small", bufs=2))
        cs_pool = stk.enter_context(tc.tile_pool(name="csp", bufs=1))
        early_stk = ExitStack()
        early = early_stk.enter_context(tc.tile_pool(name="early", bufs=1))

        if True:
            # ---- resident loads (chunked so first matmuls start early) ----
            xT_r = xT.rearrange("(dt p) r -> dt p r", p=P)
            wv_r = wv.rearrange("(dt p) v -> dt p v", p=P)
            # minimal sync triggers up front: first K-weight tile + x + wv;
            # tiny tensors go on the (idle) gpsimd SWDGE queue, late-use
            # loads are emitted at their use site
            wk_t0 = stream.tile([P, NDT, P], bf16, tag="wk", name="wk_t0",
                                bufs=4)
            xT_sb = early.tile([P, NDT, R], bf16)
            wv_sb = early.tile([P, NDT, 512], bf16)
            nc.sync.dma_start(out=wk_t0, in_=wk[0])
            nc.sync.dma_start(
                out=xT_sb, in_=xT.rearrange("(dt p) r -> p dt r", p=P))
            nc.sync.dma_start(
                out=wv_sb, in_=wv.rearrange("(dt p) v -> p dt v", p=P))
            cs_sb = res.tile([64, 2, R], f32)
            nc.sync.dma_start(out=cs_sb, in_=cs.rearrange("s p t -> p s t"))
            bvr_sb = res.tile([P, 512], f32)
            nc.sync.dma_start(out=bvr_sb, in_=bvr[:, :])
            bq_sb = res.tile([P, NQC], f32)
            nc.gpsimd.dma_start(out=bq_sb, in_=bq.rearrange("a p -> p a"))
            qn_sb = res.tile([P, NQC], f32)
            nc.gpsimd.dma_start(out=qn_sb, in_=qn.rearrange("a p -> p a"))
            bk_sb = res.tile([P, NKC], f32)
            nc.gpsimd.dma_start(out=bk_sb, in_=bk.rearrange("a p -> p a"))
            kn_sb = res.tile([P, NKC], f32)
            nc.gpsimd.dma_start(out=kn_sb, in_=kn.rearrange("a p -> p a"))
            gate_sb = res.tile([1, HQ], f32)
            nc.gpsimd.dma_start(out=gate_sb, in_=gate[:, :])
            msk_sb = res.tile([P, 4, P], bf16)
            nc.gpsimd.dma_start(out=msk_sb, in_=msk[:, :, :])

            ones_sb = res.tile([P, 1], bf16)
            nc.vector.memset(ones_sb, 1.0)
            onesrow_sb = res.tile([1, P], f32)
            nc.vector.memset(onesrow_sb, 1.0)
            eps_sb = res.tile([1, 1], f32)
            nc.vector.memset(eps_sb, RMS_EPS)

            qT = res.tile([P, NQC, R], bf16)
            kTl = res.tile([P, NKC, R], bf16)
            vl = res.tile([P, 4, R], bf16)
            kT_f = res.tile([P, HKV, 4, 512], bf16)   # [d, h4, r, i]
            v_f = res.tile([P, 4, 4, 512], bf16)      # [keys, r, w, vcol]
            attnT = res.tile([P, HQ, R], bf16)

            ps_stack = ExitStack()
            ps_proj = ps_stack.enter_context(
                tc.tile_pool(name="ps_proj", bufs=3, space="PSUM"))
            ps_small = ps_stack.enter_context(
                tc.tile_pool(name="ps_small", bufs=2, space="PSUM"))

            def finish_rms(sq_ps, inv_n):
                """[1,R] f32 = 1/sqrt(mean(col sums of squares)+eps)."""
                tmp_sb = small.tile([1, R], f32, tag="rms_tmp", bufs=1)
                nc.scalar.activation(tmp_sb, sq_ps, Sqrt,
                                     bias=eps_sb[:, 0:1], scale=inv_n)
                dst_sb = small.tile([1, R], f32, tag="rkq", bufs=1)
                nc.vector.reciprocal(dst_sb, tmp_sb)
                return dst_sb

            def build_cs(r_sb):
                """c1 = [cos*r; sin*r], c2 = [-sin*r; cos*r]."""
                rb_ps = ps_proj.tile([64, R], f32, tag="proj",
                                     name="rb_ps")
                nc.tensor.matmul(rb_ps, onesrow_sb[0:1, 0:64], r_sb,
                                 skip_group_check=True)
                rb_sb = tmp.tile([64, R], f32, tag="rb", bufs=2)
                nc.scalar.copy(rb_sb, rb_ps)
                c1 = cs_pool.tile([P, R], bf16, tag="cs1", name="cs1")
                c2 = cs_pool.tile([P, R], bf16, tag="cs2", name="cs2")
                nc.vector.tensor_tensor(c1[0:64, :], cs_sb[:, 0, :],
                                        rb_sb, Alu.mult)
                nc.vector.tensor_tensor(c1[64:128, :], cs_sb[:, 1, :],
                                        rb_sb, Alu.mult)
                nc.vector.tensor_copy(c2[0:64, :], c1[64:128, :])
                nc.vector.tensor_copy(c2[64:128, :], c1[0:64, :])
                return c1, c2

            def rope_unit(dst, c1, c2):
                """In-place rotate-half RoPE on dst [128, R] bf16.

                c1 = [C*r; S*r], c2 = [S*r; C*r].  All six products pair
                operands at matching base partitions (walrus requires it).
                """
                ta = tmp.tile([64, R], bf16, tag="ropeA", bufs=2)
                tb = tmp.tile([64, R], bf16, tag="ropeB", bufs=2)
                tc2 = tmp.tile([P, R], bf16, tag="ropeC", bufs=2)
                # top*C (base 0), bot*S (base 64)
                nc.vector.tensor_tensor(ta, dst[0:64, :], c1[0:64, :],
                                        Alu.mult)
                nc.vector.tensor_tensor(tb, dst[64:128, :], c1[64:128, :],
                                        Alu.mult)
                # top*S (base 0), bot*C (base 64)
                nc.vector.tensor_tensor(tc2[64:128, :], dst[0:64, :],
                                        c2[0:64, :], Alu.mult)
                nc.vector.tensor_tensor(dst[64:128, :], dst[64:128, :],
                                        c2[64:128, :], Alu.mult)
                nc.vector.tensor_sub(dst[0:64, :], ta, tb)
                nc.vector.tensor_add(dst[64:128, :], dst[64:128, :],
                                     tc2[64:128, :])

            # ---------------- K projection ----------------
            ksq_ps = ps_small.tile([1, R], f32, name="ksq_ps")
            k_pend = []
            for kc in range(NKC):
                if kc == 0:
                    wk_t = wk_t0
                else:
                    wk_t = stream.tile([P, NDT, P], bf16, tag="wk",
                                       name="wk_t", bufs=4)
                    nc.sync.dma_start(out=wk_t, in_=wk[kc])
                ps = ps_proj.tile([P, R], f32, tag="proj")
                for dt in range(NDT):
                    nc.tensor.matmul(ps, wk_t[:, dt, :], xT_sb[:, dt, :],
                                     start=(dt == 0), stop=(dt == NDT - 1))
                zsq = tmp.tile([P, R], bf16, tag="zsq")
                nc.scalar.activation(zsq, ps, Square,
                                     bias=bk_sb[:, kc:kc + 1], scale=1.0)
                k_pend.append((kc, zsq))
                if kc >= 1:
                    pc, pz = k_pend.pop(0)
                    nc.tensor.matmul(ksq_ps, ones_sb, pz,
                                     start=(pc == 0), stop=False,
                                     skip_group_check=True)
                nc.vector.tensor_scalar(kTl[:, kc, :], ps,
                                        bk_sb[:, kc:kc + 1],
                                        kn_sb[:, kc:kc + 1],
                                        Alu.add, Alu.mult)
            pc, pz = k_pend.pop(0)
            nc.tensor.matmul(ksq_ps, ones_sb, pz, start=False, stop=True,
                             skip_group_check=True)
            rk_sb = finish_rms(ksq_ps, 1.0 / 512)
            cs1k, cs2k = build_cs(rk_sb)
            for kc in range(NKC):
                rope_unit(kTl[:, kc, :], cs1k, cs2k)
            nc.sync.dma_start(
                out=cc_in[0:512, :].rearrange("(kc p) t -> p kc t", p=P),
                in_=kTl)

            # ---------------- V projection ----------------
            for it in range(4):
                ps = ps_proj.tile([P, 512], f32, tag="proj")
                for dt in range(NDT):
                    nc.tensor.matmul(ps, xT_sb[:, dt, it * P:(it + 1) * P],
                                     wv_sb[:, dt, :],
                                     start=(dt == 0), stop=(dt == NDT - 1))
                nc.vector.tensor_tensor(vl[:, it, :], ps, bvr_sb, Alu.add)
            nc.sync.dma_start(
                out=cc_in[512:1024, :].rearrange("(it p) t -> p it t", p=P),
                in_=vl)
            nc.gpsimd.collective_compute(
                "AllGather", Alu.bypass,
                replica_groups=[[0, 1, 2, 3], [4, 5, 6, 7]],
                ins=[cc_in[:, :]], outs=[cc_out[:, :, :]])

            # ---------------- Q projection ----------------
            qsq_ps = ps_small.tile([1, R], f32, name="qsq_ps")
            q_pend = []
            for qc in range(NQC):
                wq_t = stream.tile([P, NDT, P], bf16, tag="wq", name="wq_t",
                                   bufs=4)
                nc.sync.dma_start(out=wq_t, in_=wq[qc])
                ps = ps_proj.tile([P, R], f32, tag="proj")
                for dt in range(NDT):
                    nc.tensor.matmul(ps, wq_t[:, dt, :], xT_sb[:, dt, :],
                                     start=(dt == 0), stop=(dt == NDT - 1))
                zsq = tmp.tile([P, R], bf16, tag="zsq")
                nc.scalar.activation(zsq, ps, Square,
                                     bias=bq_sb[:, qc:qc + 1], scale=1.0)
                q_pend.append((qc, zsq))
                if qc >= 2:
                    pc, pz = q_pend.pop(0)
                    nc.tensor.matmul(qsq_ps, ones_sb, pz,
                                     start=(pc == 0), stop=False,
                                     skip_group_check=True)
                nc.vector.tensor_scalar(qT[:, qc, :], ps,
                                        bq_sb[:, qc:qc + 1],
                                        qn_sb[:, qc:qc + 1],
                                        Alu.add, Alu.mult)
            for j, (pc, pz) in enumerate(q_pend):
                nc.tensor.matmul(qsq_ps, ones_sb, pz, start=False,
                                 stop=(j == len(q_pend) - 1),
                                 skip_group_check=True)
            q_pend = []
            rq_sb = finish_rms(qsq_ps, 1.0 / D_MODEL)
            cs1q, cs2q = build_cs(rq_sb)
            # rope for q heads is emitted just-in-time inside the attention
            # loop so the DVE doesn't front-load 16 units before the first
            # attention mask op can run

            # gathered K/V -> SBUF (sync queue, after all weight triggers)
            for r4 in range(4):
                nc.sync.dma_start(
                    out=kT_f[:, :, r4, :],
                    in_=cc_out[r4, 0:512, :].rearrange("(h p) t -> p h t",
                                                       p=P))
                nc.sync.dma_start(
                    out=v_f[:, r4, :, :],
                    in_=cc_out[r4, 512:1024, :].rearrange("(w p) t -> p w t",
                                                          p=P))

            ps_stack.close()
            early_stk.close()

            # ---------------- Attention ----------------
            with ExitStack() as att_stk:
                probs_pool = att_stk.enter_context(
                    tc.tile_pool(name="probs", bufs=6))
                grecip_pool = att_stk.enter_context(
                    tc.tile_pool(name="grec", bufs=2))
                ps_sc = att_stk.enter_context(
                    tc.tile_pool(name="ps_sc", bufs=2, space="PSUM"))
                ps_at = att_stk.enter_context(
                    tc.tile_pool(name="ps_at", bufs=2, space="PSUM"))
                ps_sm = att_stk.enter_context(
                    tc.tile_pool(name="ps_sm", bufs=1, space="PSUM"))
                ps_rp = att_stk.enter_context(
                    tc.tile_pool(name="ps_rp", bufs=1, space="PSUM"))
                head_ps = {}

                def emit_scores(h, t, pair):
                    h4 = h // GROUP
                    off = P * t
                    sc_ps = ps_sc.tile([P, 2, 512], f32, name="sc_ps")
                    for i2 in range(2):
                        r4 = 2 * pair + i2
                        nc.tensor.matmul(
                            sc_ps[:, i2, off:512],
                            kT_f[:, h4, r4, off:off + P],
                            qT[:, h, off:512])
                    probs = probs_pool.tile([P, 2, 512], bf16, name="probs")
                    nc.scalar.activation(
                        probs[:, :, off:512], sc_ps[:, :, off:512],
                        Exp, bias=0.0, scale=SCALE)
                    nc.gpsimd.tensor_tensor(
                        probs[:, :, off:off + P],
                        probs[:, :, off:off + P],
                        msk_sb[:, 2 * pair:2 * pair + 2, :], Alu.mult)
                    return probs

                def emit_consume(h, t, pair, probs):
                    h4 = h // GROUP
                    off = P * t
                    if h not in head_ps:
                        head_ps[h] = (ps_at.tile([P, R], f32, name="at_ps"),
                                      ps_sm.tile([1, R], f32, name="sm_ps"))
                    at_ps, sm_ps = head_ps[h]
                    for i2 in range(2):
                        r4 = 2 * pair + i2
                        first = (t == 0 and pair == 0 and i2 == 0)
                        last = (t == NQT - 1 and pair == 1 and i2 == 1)
                        nc.tensor.matmul(
                            sm_ps[0:1, off:512], ones_sb,
                            probs[:, i2, off:512],
                            start=first, stop=last,
                            skip_group_check=True)
                        nc.tensor.matmul(
                            at_ps[:, off:512],
                            v_f[:, r4, t, h4 * P:(h4 + 1) * P],
                            probs[:, i2, off:512],
                            start=first, stop=last,
                            skip_group_check=True)
                    if not (t == NQT - 1 and pair == 1):
                        return None
                    # head epilogue part A (DVE only): 1/sum, gated
                    sums_sb = small.tile([1, R], f32, tag="sums")
                    nc.vector.tensor_copy(sums_sb, sm_ps)
                    recip_sb = small.tile([1, R], f32, tag="recip")
                    nc.vector.reciprocal(recip_sb, sums_sb)
                    nc.vector.tensor_scalar(recip_sb, recip_sb,
                                            gate_sb[0:1, h:h + 1], None,
                                            Alu.mult)
                    del head_ps[h]
                    return (h, at_ps, recip_sb)

                def emit_epilogue_b(h, at_ps, recip_sb):
                    # broadcast gated 1/sum to 128 partitions, then scale
                    rp_ps = ps_rp.tile([P, R], f32, name="rp_ps")
                    nc.tensor.matmul(rp_ps, onesrow_sb, recip_sb,
                                     skip_group_check=True)
                    grecip = grecip_pool.tile([P, R], f32, name="grecip")
                    nc.scalar.copy(grecip, rp_ps)
                    nc.vector.tensor_tensor(attnT[:, h, :], at_ps, grecip,
                                            Alu.mult)

                steps = [(h, t, pr) for h in range(HQ)
                         for t in range(NQT) for pr in range(2)]
                prev = None
                pend = []
                rope_unit(qT[:, 0, :], cs1q, cs2q)
                for i, step in enumerate(steps):
                    h, t, pr = step
                    if t == 2 and pr == 0 and h + 1 < HQ:
                        rope_unit(qT[:, h + 1, :], cs1q, cs2q)
                    pb = emit_scores(*step)
                    if prev is not None:
                        ep = emit_consume(*prev[0], prev[1])
                        if ep is not None:
                            pend.append((i + 3, ep))
                    while pend and pend[0][0] <= i:
                        emit_epilogue_b(*pend.pop(0)[1])
                    prev = (step, pb)
                ep = emit_consume(*prev[0], prev[1])
                if ep is not None:
                    pend.append((0, ep))
                for _, e in pend:
                    emit_epilogue_b(*e)

            # ---------------- Output projection ----------------
            with ExitStack() as o_stk:
                ps_o = o_stk.enter_context(
                    tc.tile_pool(name="ps_o", bufs=2, space="PSUM"))
                wo_pool = o_stk.enter_context(
                    tc.tile_pool(name="wop", bufs=2))
                outsb_pool = o_stk.enter_context(
                    tc.tile_pool(name="outsb", bufs=4))
                bor_sb = res.tile([P, 4, 512], f32)
                nc.sync.dma_start(
                    out=bor_sb, in_=bor.rearrange("p (o c) -> p o c", c=512))
                for oc in range(4):
                    wo_t = wo_pool.tile([P, NDT, 512], bf16, tag="wo")
                    nc.sync.dma_start(
                        out=wo_t,
                        in_=wo[:, oc * 512:(oc + 1) * 512].rearrange(
                            "(dt p) o -> p dt o", p=P))
                    ps_list = [ps_o.tile([P, 512], f32, tag=f"o{rt}",
                                         name=f"ops{rt}")
                               for rt in range(NQT)]
                    for rt in range(NQT):
                        for hd in range(HQ):
                            nc.tensor.matmul(
                                ps_list[rt], attnT[:, hd, rt * P:(rt + 1) * P],
                                wo_t[:, hd, :],
                                start=(hd == 0), stop=(hd == HQ - 1),
                                skip_group_check=True)
                    for rt in range(NQT):
                        o_sb = outsb_pool.tile([P, 512], f32)
                        nc.vector.tensor_tensor(o_sb, ps_list[rt],
                                                bor_sb[:, oc, :], Alu.add)
                        nc.sync.dma_start(
                            out=out[rt * P:(rt + 1) * P,
                                    oc * 512:(oc + 1) * 512],
                            in_=o_sb)

    nc.compile()
    return nc


_NC_CACHE = None


def _get_nc():
    global _NC_CACHE
    if _NC_CACHE is None:
        _NC_CACHE = _build_nc()
    return _NC_CACHE


# ---------------------------------------------------------------------------
# Host-side preparation
# ---------------------------------------------------------------------------

_PREP_CACHE = {}


def _perm():
    """Even/odd -> [evens; odds] permutation within each 128-dim head."""
    p = np.arange(HEAD_DIM).reshape(64, 2).T.reshape(-1)  # [0,2,..,126,1,3,..]
    return p


def _prep_static(Wq, bq, Wk, bk, Wv, bv, Wo, bo, qn_w, kn_w, gate_logits):
    """Weight-dependent, call-invariant prep (cached by array ids)."""
    global _BF16
    key = tuple(id(a) for a in (Wq, bq, Wk, bk, Wv, bv, Wo, bo, qn_w, kn_w,
                                gate_logits))
    hit = _PREP_CACHE.get(key)
    if hit is not None:
        return hit
    import ml_dtypes
    _BF16 = ml_dtypes.bfloat16

    pm = _perm()
    qperm = (np.arange(HQ)[:, None] * HEAD_DIM + pm[None, :]).reshape(-1)
    kperm = (np.arange(HKV)[:, None] * HEAD_DIM + pm[None, :]).reshape(-1)

    Wq_p = np.ascontiguousarray(Wq[:, qperm])
    Wk_p = np.ascontiguousarray(Wk[:, kperm])
    # [qc][p (d within tile)][dt][q (col within tile)]
    wq_t = np.ascontiguousarray(
        Wq_p.astype(_BF16).reshape(NDT, P, NQC, P).transpose(2, 1, 0, 3))
    wk_t = np.ascontiguousarray(
        Wk_p.astype(_BF16).reshape(NDT, P, NKC, P).transpose(2, 1, 0, 3))
    wv_b = np.ascontiguousarray(Wv.astype(_BF16))
    wo_b = np.ascontiguousarray(Wo.astype(_BF16))

    bq_p = np.ascontiguousarray(bq[qperm].astype(np.float32).reshape(NQC, P))
    qn_p = np.ascontiguousarray(qn_w[qperm].astype(np.float32).reshape(NQC, P))
    bk_p = np.ascontiguousarray(bk[kperm].astype(np.float32).reshape(NKC, P))
    kn_p = np.ascontiguousarray(kn_w[kperm].astype(np.float32).reshape(NKC, P))
    bvr = np.ascontiguousarray(
        np.broadcast_to(bv.astype(np.float32)[None, :], (P, 512)))
    bor = np.ascontiguousarray(
        np.broadcast_to(bo.astype(np.float32)[None, :], (P, D_MODEL)))
    gates = 1.0 / (1.0 + np.exp(-gate_logits.astype(np.float32)))
    gate_t = np.ascontiguousarray(gates.reshape(1, HQ).astype(np.float32))

    ent = dict(wq=wq_t, wk=wk_t, wv=wv_b, wo=wo_b, bq=bq_p, qn=qn_p,
               bk=bk_p, kn=kn_p, bvr=bvr, bor=bor, gate=gate_t,
               _refs=(Wq, bq, Wk, bk, Wv, bv, Wo, bo, qn_w, kn_w, gate_logits))
    _PREP_CACHE.clear()
    _PREP_CACHE[key] = ent
    return ent


def _cs_table(c, sp):
    """cos/sin for positions sp + 4*i + c, i in 0..R-1. [2, 64, R] f32."""
    inv = 1.0 / (ROPE_THETA ** (np.arange(64, dtype=np.float64) / 64.0))
    pos = sp + 4.0 * np.arange(R, dtype=np.float64) + c
    ang = pos[None, :] * inv[:, None]              # [64, R]
    return np.ascontiguousarray(
        np.stack([np.cos(ang), np.sin(ang)]).astype(np.float32))


def _mask_tiles(c):
    """msk[k, r, j] = 1 if key (4k+r) <= query (4j+c) else 0; bf16."""
    k = np.arange(P)[:, None, None]
    r = np.arange(4)[None, :, None]
    j = np.arange(P)[None, None, :]
    return np.ascontiguousarray(
        ((4 * k + r) <= (4 * j + c)).astype(_BF16))


def _is_tril(mask):
    m = np.asarray(mask)
    if m.shape != (S, S):
        return False
    idx = np.arange(S)
    # spot-check: full check is 4M elements, cheap enough once
    return bool(np.array_equal(m, idx[None, :] <= idx[:, None]))


_TRIL_CACHE = {}


# ---------------------------------------------------------------------------
# Fallback implementations (numpy / jax.jit)
# ---------------------------------------------------------------------------

def _np_rmsnorm(x, w):
    var = np.mean(np.square(x), axis=-1, keepdims=True)
    return x * (1.0 / np.sqrt(var + RMS_EPS)) * w


def _np_rope(x, positions):
    half = x.shape[-1] // 2
    inv_freq = 1.0 / (ROPE_THETA ** (np.arange(half, dtype=np.float32) / half))
    ang = positions.astype(np.float32)[:, None] * inv_freq[None, :]
    cos, sin = np.cos(ang), np.sin(ang)
    while cos.ndim < x.ndim:
        cos, sin = cos[None], sin[None]
    x1, x2 = x[..., 0::2], x[..., 1::2]
    outp = np.empty_like(x)
    outp[..., 0::2] = x1 * cos - x2 * sin
    outp[..., 1::2] = x1 * sin + x2 * cos
    return outp


def _np_reference(x, Wq, bq, Wk, bk, Wv, bv, Wo, bo, qn_w, kn_w,
                  gate_logits, mask, sp):
    outp = np.empty((B, S, D_MODEL), dtype=np.float32)
    positions = sp + np.arange(S)
    gates = 1.0 / (1.0 + np.exp(-gate_logits.astype(np.float32)))
    for b in range(B):
        q = _np_rmsnorm(x[b] @ Wq + bq, qn_w)
        k = _np_rmsnorm(x[b] @ Wk + bk, kn_w)
        v = x[b] @ Wv + bv
        q = q.reshape(S, HQ, HEAD_DIM).transpose(1, 0, 2)
        k = k.reshape(S, HKV, HEAD_DIM).transpose(1, 0, 2)
        v = v.reshape(S, HKV, HEAD_DIM).transpose(1, 0, 2)
        q = _np_rope(q, positions)
        k = _np_rope(k, positions)
        attn = np.empty((S, HQ, HEAD_DIM), dtype=np.float32)
        for h in range(HQ):
            g = h // GROUP
            s = (q[h] @ k[g].T) * SCALE
            s = np.where(mask, s, -np.inf).astype(np.float32)
            s -= s.max(axis=-1, keepdims=True)
            p = np.exp(s)
            p /= p.sum(axis=-1, keepdims=True)
            attn[:, h, :] = (p @ v[g]) * gates[h]
        outp[b] = attn.reshape(S, D_MODEL) @ Wo + bo
    return outp


# ---------------------------------------------------------------------------
# Entry point
# ---------------------------------------------------------------------------

LAST_EXEC_NS = None


def kernel(x, Wq, bq, Wk, bk, Wv, bv, Wo, bo, qn_w, kn_w,
           gate_logits, mask, start_pos, **_ignored):
    global LAST_EXEC_NS
    x = np.asarray(x, dtype=np.float32)
    Wq = np.asarray(Wq, dtype=np.float32); bq = np.asarray(bq, dtype=np.float32)
    Wk = np.asarray(Wk, dtype=np.float32); bk = np.asarray(bk, dtype=np.float32)
    Wv = np.asarray(Wv, dtype=np.float32); bv = np.asarray(bv, dtype=np.float32)
    Wo = np.asarray(Wo, dtype=np.float32); bo = np.asarray(bo, dtype=np.float32)
    qn_w = np.asarray(qn_w, dtype=np.float32)
    kn_w = np.asarray(kn_w, dtype=np.float32)
    gate_logits = np.asarray(gate_logits, dtype=np.float32)
    mask = np.asarray(mask)
    sp = int(np.asarray(start_pos))

    if not os.environ.get("GQA_NO_DEVICE"):
        mk = id(mask)
        if mk not in _TRIL_CACHE:
            _TRIL_CACHE.clear()
            _TRIL_CACHE[mk] = (_is_tril(mask), mask)
        if _TRIL_CACHE[mk][0] and x.shape == (B, S, D_MODEL):
            try:
                return _bass_path(x, Wq, bq, Wk, bk, Wv, bv, Wo, bo,
                                  qn_w, kn_w, gate_logits, sp)
            except Exception:
                import traceback
                traceback.print_exc()

    return _np_reference(x, Wq, bq, Wk, bk, Wv, bv, Wo, bo, qn_w, kn_w,
                         gate_logits, mask, sp)


def _bass_path(x, Wq, bq, Wk, bk, Wv, bv, Wo, bo, qn_w, kn_w,
               gate_logits, sp):
    global LAST_EXEC_NS
    from concourse.bass_utils import run_bass_kernel_spmd

    st = _prep_static(Wq, bq, Wk, bk, Wv, bv, Wo, bo, qn_w, kn_w, gate_logits)
    static = {k: v for k, v in st.items() if not k.startswith("_")}

    in_maps = []
    for core in range(N_CORES):
        b, c = divmod(core, 4)
        xT_own = np.ascontiguousarray(x[b, c::4, :].T.astype(_BF16))
        m = dict(static)
        m["xT"] = xT_own
        m["cs"] = _cs_table(c, sp)
        m["msk"] = _mask_tiles(c)
        in_maps.append(m)

    trace = bool(os.environ.get("GQA_TRACE"))
    try:
        res = run_bass_kernel_spmd(_get_nc(), in_maps,
                                   core_ids=list(range(N_CORES)), trace=trace)
    except Exception:
        if not trace:
            raise
        res = run_bass_kernel_spmd(_get_nc(), in_maps,
                                   core_ids=list(range(N_CORES)), trace=False)
    LAST_EXEC_NS = res.exec_time_ns

    outp = np.empty((B, S, D_MODEL), dtype=np.float32)
    for core in range(N_CORES):
        b, c = divmod(core, 4)
        outp[b, c::4, :] = res.results[core]["out"]
    return outp
